# revision 27
# baseline (speedup 1.0000x reference)
"""Trainium2 Bass kernel for beam-search top-k masking (nn_Beam_57612691308621).

Strategy: shard the prompt dim P=32 across 8 NeuronCores (4 prompts each).
Each core, fully on-device:
  1. Streams its (32 rows x 128000) probs shard through SBUF in chunks,
     computing per-128-element segment maxima (the only full-data pass),
     transposed per-chunk into a per-beam-row layout.
  2. Selects top-16 segments per beam row (max8/max_index/match_replace),
     re-gathers those segments from HBM via indirect DMA, and extracts the
     exact per-beam top-16 prob values + vocab indices.
  3. Computes log-probs (ScalarE Ln) for the 16 candidates/beam only
     (log is monotone, so per-beam ordering by prob == ordering by logprob),
     then does the joint (beam, vocab) top-16 per prompt, the first-step
     override, EOS masking, grow-alive / grow-fin top-8, gathers the output
     sequence rows on device, and writes the new token at cur_pos via a
     bounds-checked indirect scatter.
Host only shards inputs / concatenates outputs.
"""

import os
import sys

for _p in ("/opt/trn_rl_repo", "/root/.axon_site", "/root/.axon_site/_ro/trn_rl_repo",
           "/root/.axon_site/_ro/pypackages"):
    if os.path.isdir(_p) and _p not in sys.path:
        sys.path.append(_p)

import numpy as np

import concourse.bass as bass
import concourse.bacc as bacc
import concourse.mybir as mybir
from concourse import tile

dt = mybir.dt
AF = mybir.ActivationFunctionType
ALU = mybir.AluOpType
AX = mybir.AxisListType

N_CORES = 8
P_FULL, D, V, S = 32, 8, 128000, 2048
Pc = P_FULL // N_CORES          # prompts per core = 4
R = Pc * D                      # beam rows per core = 32
Q = 4                           # row quarters -> R*Q = 128 partitions
Vq = V // Q                     # 32000 elems per partition
W = 128                         # segment width
NSEG = Vq // W                  # 250 segments per partition
NSEG_ROW = V // W               # 1000 segments per beam row
NCHUNK = 10
CW = Vq // NCHUNK               # 3200 elems per chunk per partition
SEG_PER_CHUNK = CW // W         # 25
K2 = 16                         # 2*D candidates
EOS = 2
INF = 1.0e7
NEG = -3.0e38
OOB = 10 ** 9                   # scatter offset sentinel (dropped by bounds check)


def build_core_program(nc, cur_pos):
    f32, i32, u32, u8 = dt.float32, dt.int32, dt.uint32, dt.uint8

    probs = nc.dram_tensor("probs", (R, V), f32, kind="ExternalInput")
    seqs = nc.dram_tensor("seqs", (2 * R, S), i32, kind="ExternalInput")
    alive_lp = nc.dram_tensor("alive_lp", (Pc, D), f32, kind="ExternalInput")
    fin_lp = nc.dram_tensor("fin_lp", (Pc, D), f32, kind="ExternalInput")
    sp_in = nc.dram_tensor("sp", (Pc, 1), f32, kind="ExternalInput")
    isf_in = nc.dram_tensor("isf", (Pc, 1), f32, kind="ExternalInput")

    att_out = nc.dram_tensor("att", (Pc, D), i32, kind="ExternalOutput")
    na_seq_out = nc.dram_tensor("na_seq", (R, S), i32, kind="ExternalOutput")
    na_lp_out = nc.dram_tensor("na_lp", (Pc, D), f32, kind="ExternalOutput")
    nf_seq_out = nc.dram_tensor("nf_seq", (R, S), i32, kind="ExternalOutput")
    nf_lp_out = nc.dram_tensor("nf_lp", (Pc, D), f32, kind="ExternalOutput")

    with tile.TileContext(nc) as tc:
        with (
            tc.tile_pool(name="stream", bufs=4) as stream_pool,
            tc.tile_pool(name="work", bufs=1) as wp,
            tc.tile_pool(name="stage", bufs=1) as sgp,
        ):
            # ------- phase 1 first: keep the SP HWDGE queue clear for streaming
            probs_t = probs.ap().rearrange("r (q v) -> (r q) v", q=Q)  # (128, 32000)
            segmax = wp.tile([128, NSEG], f32)
            segrow = wp.tile([R, NSEG_ROW], f32)    # per-beam-row segment maxima
            for c in range(NCHUNK):
                chunk = stream_pool.tile([128, CW], f32, tag="chunk")
                nc.sync.dma_start(chunk[:], probs_t[:, c * CW:(c + 1) * CW])
                seg_sl = segmax[:, c * SEG_PER_CHUNK:(c + 1) * SEG_PER_CHUNK]
                nc.vector.reduce_max(
                    seg_sl.unsqueeze(2),
                    chunk[:].rearrange("p (s w) -> p s w", w=W),
                    axis=AX.X,
                )
                # transpose this chunk's maxima into row-major layout (ACT queue)
                dst = segrow[:].rearrange("r (q s) -> r q s", q=Q)[
                    :, :, c * SEG_PER_CHUNK:(c + 1) * SEG_PER_CHUNK]
                nc.scalar.dma_start(dst, seg_sl)

            # ---------------- constants (inline tables, ACT queue) -----------
            rid = np.arange(R)
            c32_np = np.concatenate([
                rid[:, None],                                   # 0: row id
                (rid * NSEG_ROW)[:, None],                      # 1: row seg base
                np.repeat(np.arange(K2)[None, :], R, axis=0),   # 2..17: slot iota
                (rid % D)[:, None],                             # 18: beam-slot d
                ((rid // D) * D)[:, None],                      # 19: prompt row base
                np.ones((R, 1)),                                # 20: ones
            ], axis=1).astype(np.float32)                       # (32, 21)
            c32_dram = nc.inline_tensor(c32_np, name="c32")
            c32 = wp.tile([R, 5 + K2], f32)
            nc.scalar.dma_start(c32[:], c32_dram.ap())
            rowid_f = c32[:, 0:1]
            rowbase_f = c32[:, 1:2]
            islot_f = c32[:, 2:2 + K2]
            dmod_f = c32[:, 2 + K2:3 + K2]
            rowbase8_f = c32[:, 3 + K2:4 + K2]
            ones32 = c32[:, 4 + K2:5 + K2]

            c64_np = np.repeat(np.arange(D * K2)[None, :], 2 * R, axis=0).astype(np.float32)
            c64_dram = nc.inline_tensor(c64_np, name="c64")
            c64 = wp.tile([2 * R, D * K2], f32)
            nc.scalar.dma_start(c64[:], c64_dram.ap())

            # ---------------- small input loads (ACT queue) ------------------
            alive32 = wp.tile([R, 1], f32)          # alive_lp per beam row
            nc.scalar.dma_start(alive32[:], alive_lp.ap().rearrange("p d -> (p d)").unsqueeze(1))
            alive8 = wp.tile([Pc, D], f32)
            nc.scalar.dma_start(alive8[:], alive_lp.ap())
            fin8 = wp.tile([Pc, D], f32)
            nc.scalar.dma_start(fin8[:], fin_lp.ap())
            sp1 = wp.tile([Pc, 1], f32)
            nc.scalar.dma_start(sp1[:], sp_in.ap())
            isf1 = wp.tile([Pc, 1], f32)
            nc.scalar.dma_start(isf1[:], isf_in.ap())
            sp_u8 = wp.tile([Pc, 1], u8)
            nc.vector.tensor_copy(sp_u8[:], sp1[:])
            isf_u8 = wp.tile([Pc, 1], u8)
            nc.vector.tensor_copy(isf_u8[:], isf1[:])

            # ---------------- phase 2: top-16 segments per beam row ----------
            segv = wp.tile([R, K2], f32)
            segidx = wp.tile([R, K2], u32)
            sm2 = wp.tile([R, NSEG_ROW], f32)
            nc.vector.max(out=segv[:, 0:8], in_=segrow[:])
            nc.vector.max_index(out=segidx[:, 0:8], in_max=segv[:, 0:8], in_values=segrow[:])
            nc.vector.match_replace(out=sm2[:], in_to_replace=segv[:, 0:8],
                                    in_values=segrow[:], imm_value=NEG)
            nc.vector.max(out=segv[:, 8:16], in_=sm2[:])
            nc.vector.max_index(out=segidx[:, 8:16], in_max=segv[:, 8:16], in_values=sm2[:])

            lseg_f = wp.tile([R, K2], f32)          # per-row local seg id (0..999)
            gseg_f = wp.tile([R, K2], f32)          # global seg id = row*1000 + local
            gseg_i = wp.tile([R, K2], i32)
            for h in range(2):
                hs = slice(8 * h, 8 * h + 8)
                nc.vector.tensor_copy(lseg_f[:, hs], segidx[:, hs])
                nc.vector.tensor_scalar(gseg_f[:, hs], lseg_f[:, hs],
                                        rowbase_f[:, 0:1], None, op0=ALU.add)
                nc.vector.tensor_copy(gseg_i[:, hs], gseg_f[:, hs])

            # ---------------- phase 3: drill-down gather + exact row top-16 --
            # distribute the 512 segment fetches over all 128 partitions:
            # partition (r, g) on call c fetches the row's slot s = c*4+g
            # partition (r,g) on call c fetches rank slot s = c*4+g, so calls 0/1
            # only need the first max8 round's results (pipelines with round 2)
            gsegP = wp.tile([R, K2], i32)           # [g*4+c] = gseg[c*4+g]
            gsegT = wp.tile([4 * R, Q], i32)
            gathQ = wp.tile([4 * R, Q * W], f32)
            gath = wp.tile([R, K2 * W], f32)        # [r, (g*4+c)*W + w]
            gath_v = gath[:].rearrange("r (g c w) -> r g c w", g=Q, c=Q)
            probs_seg = probs.ap().rearrange("r (s w) -> (r s) w", w=W)
            gsegP_v = gsegP[:].rearrange("r (g c) -> r g c", g=Q)
            gseg_v = gseg_i[:].rearrange("r (c g) -> r g c", g=Q)
            for h in range(2):
                nc.vector.tensor_copy(gsegP_v[:, :, 2 * h:2 * h + 2],
                                      gseg_v[:, :, 2 * h:2 * h + 2])
                nc.scalar.dma_start(
                    gsegT[:, 2 * h:2 * h + 2],
                    gsegP_v[:, :, 2 * h:2 * h + 2])
                for c in (2 * h, 2 * h + 1):
                    nc.gpsimd.indirect_dma_start(
                        out=gathQ[:, c * W:(c + 1) * W],
                        out_offset=None,
                        in_=probs_seg,
                        in_offset=bass.IndirectOffsetOnAxis(ap=gsegT[:, c:c + 1], axis=0),
                    )
                    nc.sync.dma_start(gath_v[:, :, c, :],
                                      gathQ[:, c * W:(c + 1) * W])
            bv = wp.tile([R, K2], f32)              # per-beam-row top-16 prob values
            qpos = wp.tile([R, K2], u32)            # positions in gath (0..2047)
            g2 = wp.tile([R, K2 * W], f32)
            nc.vector.max(out=bv[:, 0:8], in_=gath[:])
            nc.vector.max_index(out=qpos[:, 0:8], in_max=bv[:, 0:8], in_values=gath[:])
            nc.vector.match_replace(out=g2[:], in_to_replace=bv[:, 0:8],
                                    in_values=gath[:], imm_value=NEG)
            nc.vector.max(out=bv[:, 8:16], in_=g2[:])
            nc.vector.max_index(out=qpos[:, 8:16], in_max=bv[:, 8:16], in_values=g2[:])

            slot_u = wp.tile([R, K2], u32)
            nc.vector.tensor_scalar(slot_u[:], qpos[:], 7, None, op0=ALU.logical_shift_right)
            off_u = wp.tile([R, K2], u32)
            nc.vector.tensor_scalar(off_u[:], qpos[:], W - 1, None, op0=ALU.bitwise_and)
            slot_f = wp.tile([R, K2], f32)
            nc.vector.tensor_copy(slot_f[:], slot_u[:])
            off_f = wp.tile([R, K2], f32)
            nc.vector.tensor_copy(off_f[:], off_u[:])

            # local seg ids keyed by gath slot position (rank slot c*4+g)
            lperm_f = wp.tile([R, K2], f32)
            nc.vector.tensor_copy(
                lperm_f[:].rearrange("r (g c) -> r g c", g=Q),
                lseg_f[:].rearrange("r (c g) -> r g c", g=Q))
            # one-hot over the 16 slot positions
            eq3 = wp.tile([R, K2 * K2], f32)
            eq3v = eq3[:].rearrange("p (c s) -> p c s", s=K2)
            nc.vector.tensor_tensor(
                eq3v,
                slot_f[:].unsqueeze(2).broadcast_to([R, K2, K2]),
                islot_f.unsqueeze(1).broadcast_to([R, K2, K2]),
                op=ALU.is_equal,
            )
            nc.vector.tensor_tensor(
                eq3v, eq3v,
                lperm_f[:].unsqueeze(1).broadcast_to([R, K2, K2]),
                op=ALU.mult,
            )
            lcand_f = wp.tile([R, K2], f32)
            nc.vector.reduce_sum(lcand_f[:].unsqueeze(2), eq3v, axis=AX.X)

            btok = wp.tile([R, K2], f32)            # vocab index per beam candidate
            nc.vector.tensor_scalar(btok[:], lcand_f[:], float(W), None, op0=ALU.mult)
            nc.vector.tensor_add(btok[:], btok[:], off_f[:])

            # ---------------- phase 5: log + joint top-16 per prompt ---------
            logp = wp.tile([R, K2], f32)
            nc.scalar.activation(logp[:], bv[:], AF.Ln)
            curr = wp.tile([R, K2], f32)
            nc.vector.tensor_scalar(curr[:], logp[:], alive32[:, 0:1], None, op0=ALU.add)

            currp = wp.tile([Pc, D * K2], f32)      # (4, 128) joint candidates
            nc.sync.dma_start(currp[:].rearrange("p (d c) -> p d c", d=D), curr[:])
            tokp = wp.tile([Pc, D * K2], f32)
            nc.sync.dma_start(tokp[:].rearrange("p (d c) -> p d c", d=D), btok[:])

            jv = wp.tile([Pc, K2], f32)             # joint top-16 logprob values
            jpos = wp.tile([Pc, K2], u32)
            j2 = wp.tile([Pc, D * K2], f32)
            nc.vector.max(out=jv[:, 0:8], in_=currp[:])
            nc.vector.max_index(out=jpos[:, 0:8], in_max=jv[:, 0:8], in_values=currp[:])
            nc.vector.match_replace(out=j2[:], in_to_replace=jv[:, 0:8],
                                    in_values=currp[:], imm_value=NEG)
            nc.vector.max(out=jv[:, 8:16], in_=j2[:])
            nc.vector.max_index(out=jpos[:, 8:16], in_max=jv[:, 8:16], in_values=j2[:])

            jpos_f = wp.tile([Pc, K2], f32)
            nc.vector.tensor_copy(jpos_f[:], jpos[:])
            beam_u = wp.tile([Pc, K2], u32)
            nc.vector.tensor_scalar(beam_u[:], jpos[:], 4, None, op0=ALU.logical_shift_right)
            bj = wp.tile([Pc, 2 * K2], f32)         # [beam(16) | jtok(16)]
            beam_f = bj[:, 0:K2]                    # parent beam of each topk entry
            nc.vector.tensor_copy(beam_f, beam_u[:])

            # token of each joint-topk entry: one-hot dot in (64,1) layout
            jposT = wp.tile([2 * R, 1], f32)
            nc.scalar.dma_start(jposT[:], jpos_f[:])
            tokpT = wp.tile([2 * R, D * K2], f32)
            nc.scalar.dma_start(
                tokpT[:], tokp[:].unsqueeze(1).broadcast_to([Pc, K2, D * K2]))
            junk64 = wp.tile([2 * R, D * K2], f32)
            jtokT = wp.tile([2 * R, 1], f32)
            nc.vector.scalar_tensor_tensor(
                out=junk64[:], in0=c64[:], scalar=jposT[:, 0:1], in1=tokpT[:],
                op0=ALU.is_equal, op1=ALU.mult, accum_out=jtokT[:])
            jtok = bj[:, K2:2 * K2]                 # token of each topk entry
            nc.scalar.dma_start(jtok, jtokT[:])

            # ---------------- phase 6: first-generation override -------------
            isfb = isf_u8[:, 0:1].broadcast_to([Pc, K2])
            nc.vector.copy_predicated(jv[:], isfb, currp[:, 0:K2])
            nc.vector.copy_predicated(jtok, isfb, tokp[:, 0:K2])

            # ---------------- phase 7: grow_alive / grow_fin -----------------
            fino = wp.tile([Pc, K2], f32)           # finished = tok == EOS
            nc.vector.tensor_scalar(fino[:], jtok, float(EOS), None, op0=ALU.is_equal)

            am = wp.tile([Pc, K2], f32)             # alive-masked = jv + fino*(-INF)
            nc.vector.scalar_tensor_tensor(
                out=am[:], in0=fino[:], scalar=-INF, in1=jv[:],
                op0=ALU.mult, op1=ALU.add)

            nav = wp.tile([Pc, D], f32)
            np2 = wp.tile([Pc, 2 * D], u32)         # [nap(8) | nfp(8)]
            nc.vector.max(out=nav[:], in_=am[:])
            nc.vector.max_index(out=np2[:, 0:2 * D:2], in_max=nav[:], in_values=am[:])

            spb8 = sp_u8[:, 0:1].broadcast_to([Pc, D])

            # new_alive_lp = where(sp, alive_lp, nav)
            nalp = wp.tile([Pc, D], f32)
            nc.vector.tensor_copy(nalp[:], nav[:])
            nc.vector.copy_predicated(nalp[:], spb8, alive8[:])
            nc.sync.dma_start(na_lp_out.ap(), nalp[:])

            # ----- fin side (per-prompt layout) -----
            notf = wp.tile([Pc, K2], f32)
            nc.vector.tensor_scalar(notf[:], fino[:], -1.0, 1.0, op0=ALU.mult, op1=ALU.add)
            fm = wp.tile([Pc, K2], f32)             # fin-masked = jv + (1-fino)*(-INF)
            nc.vector.scalar_tensor_tensor(
                out=fm[:], in0=notf[:], scalar=-INF, in1=jv[:],
                op0=ALU.mult, op1=ALU.add)

            cat = wp.tile([Pc, D + K2], f32)        # [fin_lp(8) | fin_masked(16)]
            nc.vector.tensor_copy(cat[:, 0:D], fin8[:])
            nc.vector.tensor_copy(cat[:, D:D + K2], fm[:])

            nfv = wp.tile([Pc, D], f32)
            nc.vector.max(out=nfv[:], in_=cat[:])
            nc.vector.max_index(out=np2[:, 1:2 * D:2], in_max=nfv[:], in_values=cat[:])
            np2_f = wp.tile([Pc, 2 * D], f32)
            nc.vector.tensor_copy(np2_f[:], np2[:])

            # new_fin_lp = where(sp, fin_lp, nfv)
            nflp = wp.tile([Pc, D], f32)
            nc.vector.tensor_copy(nflp[:], nfv[:])
            nc.vector.copy_predicated(nflp[:], spb8, fin8[:])
            nc.sync.dma_start(nf_lp_out.ap(), nflp[:])

            # ----- per-(prompt, slot) layout: one output row per partition -----
            npT = wp.tile([R, 2], f32)              # [nap | nfp] per row
            nc.scalar.dma_start(npT[:],
                                np2_f[:].rearrange("p (d f) -> p d f", f=2))
            napT = npT[:, 0:1]
            nfpT = npT[:, 1:2]
            bjT = wp.tile([R, 2 * K2], f32)         # [beam(16) | jtok(16)] per row
            nc.scalar.dma_start(
                bjT[:], bj[:].unsqueeze(1).broadcast_to([Pc, D, 2 * K2]))
            beamT = bjT[:, 0:K2]
            jtokrT = bjT[:, K2:2 * K2]
            spT = wp.tile([R, 1], f32)
            nc.scalar.dma_start(
                spT[:], sp1[:].unsqueeze(1).broadcast_to([Pc, D, 1]))
            spT_u8 = wp.tile([R, 1], u8)
            nc.vector.tensor_copy(spT_u8[:], spT[:])

            junk32 = wp.tile([R, K2], f32)
            attT = wp.tile([R, 1], f32)             # selected parent beam per row
            nc.vector.scalar_tensor_tensor(
                out=junk32[:], in0=islot_f, scalar=napT, in1=beamT,
                op0=ALU.is_equal, op1=ALU.mult, accum_out=attT[:])
            atokT = wp.tile([R, 1], f32)            # new token per alive row
            nc.vector.scalar_tensor_tensor(
                out=junk32[:], in0=islot_f, scalar=napT, in1=jtokrT,
                op0=ALU.is_equal, op1=ALU.mult, accum_out=atokT[:])

            # attention_change_ids = where(sp, d, att)
            nc.vector.copy_predicated(attT[:], spT_u8[:], dmod_f)
            att_i = wp.tile([R, 1], i32)
            nc.vector.tensor_copy(att_i[:], attT[:])
            nc.sync.dma_start(att_out.ap(), att_i[:])

            # alive source row + token + write flag
            arow_f = wp.tile([R, 1], f32)
            nc.vector.tensor_scalar(arow_f[:], attT[:], rowbase8_f[:, 0:1], None,
                                    op0=ALU.add)
            notspT = wp.tile([R, 1], f32)
            nc.vector.tensor_scalar(notspT[:], spT[:], -1.0, 1.0,
                                    op0=ALU.mult, op1=ALU.add)

            # fin old/new resolution per row
            isoldT = wp.tile([R, 1], f32)           # nfp < 8 (or sp)
            nc.vector.tensor_scalar(isoldT[:], nfpT, float(D), None, op0=ALU.is_lt)
            nc.vector.copy_predicated(isoldT[:], spT_u8[:], ones32)
            isoldT_u8 = wp.tile([R, 1], u8)
            nc.vector.tensor_copy(isoldT_u8[:], isoldT[:])
            oldrowT = wp.tile([R, 1], f32)
            nc.vector.tensor_scalar(oldrowT[:], nfpT, float(D - 1), None, op0=ALU.min)
            nc.vector.copy_predicated(oldrowT[:], spT_u8[:], dmod_f)
            candT = wp.tile([R, 1], f32)            # clamp(nfp-8, 0)
            nc.vector.tensor_scalar(candT[:], nfpT, float(D), 0.0,
                                    op0=ALU.subtract, op1=ALU.max)

            fbeamT = wp.tile([R, 1], f32)
            nc.vector.scalar_tensor_tensor(
                out=junk32[:], in0=islot_f, scalar=candT[:, 0:1], in1=beamT,
                op0=ALU.is_equal, op1=ALU.mult, accum_out=fbeamT[:])
            ftokT = wp.tile([R, 1], f32)
            nc.vector.scalar_tensor_tensor(
                out=junk32[:], in0=islot_f, scalar=candT[:, 0:1], in1=jtokrT,
                op0=ALU.is_equal, op1=ALU.mult, accum_out=ftokT[:])

            # fin source row: old -> R + base + oldrow ; new -> base + fbeam
            frow_f = wp.tile([R, 1], f32)
            nc.vector.tensor_scalar(frow_f[:], fbeamT[:], rowbase8_f[:, 0:1], None,
                                    op0=ALU.add)
            frow_old = wp.tile([R, 1], f32)
            nc.vector.tensor_scalar(frow_old[:], oldrowT[:], float(R),
                                    rowbase8_f[:, 0:1], op0=ALU.add, op1=ALU.add)
            nc.vector.copy_predicated(frow_f[:], isoldT_u8[:], frow_old[:])

            # fin write flag = !sp & !isold
            wf_f = wp.tile([R, 1], f32)
            nc.vector.scalar_tensor_tensor(
                out=wf_f[:], in0=isoldT[:], scalar=0.0, in1=notspT[:],
                op0=ALU.is_equal, op1=ALU.mult)

            # ---------------- seq gathers + token writes + outputs -----------
            # combined offsets/tokens/flags on 64 partitions: [alive rows | fin rows]
            offs = wp.tile([2 * R, 1], i32)
            nc.vector.tensor_copy(offs[0:R, :], arow_f[:])
            nc.vector.tensor_copy(offs[R:2 * R, :], frow_f[:])
            toks = wp.tile([2 * R, 1], i32)
            nc.vector.tensor_copy(toks[0:R, :], atokT[:])
            nc.vector.tensor_copy(toks[R:2 * R, :], ftokT[:])
            wmask = wp.tile([2 * R, 1], u8)
            nc.vector.tensor_copy(wmask[0:R, :], notspT[:])
            nc.vector.tensor_copy(wmask[R:2 * R, :], wf_f[:])

            stage = sgp.tile([2 * R, S], i32)
            nc.gpsimd.indirect_dma_start(
                out=stage[:], out_offset=None, in_=seqs.ap(),
                in_offset=bass.IndirectOffsetOnAxis(ap=offs[:], axis=0))

            # write new token at the cur_pos column of the staged rows
            # (cur_pos is specialized at compile time; recompiled if it changes)
            cp = int(cur_pos)
            nc.vector.copy_predicated(stage[:, cp:cp + 1], wmask[:], toks[:])

            nc.sync.dma_start(na_seq_out.ap(), stage[0:R, :])
            nc.sync.dma_start(nf_seq_out.ap(), stage[R:2 * R, :])

    return nc


_CACHED_NC = {}


def _get_nc(cur_pos=1024):
    cp = int(cur_pos)
    if cp not in _CACHED_NC:
        nc = bacc.Bacc("TRN2", target_bir_lowering=False, debug=False)
        build_core_program(nc, cp)
        nc.compile()
        _CACHED_NC[cp] = nc
    return _CACHED_NC[cp]


def make_in_maps(probs, alive_seq, fin_seq, alive_log_probs, fin_log_probs,
                 still_prompt, is_first, cur_pos):
    probs = np.asarray(probs, dtype=np.float32).reshape(P_FULL, D, V)
    alive_seq = np.asarray(alive_seq, dtype=np.int32)
    fin_seq = np.asarray(fin_seq, dtype=np.int32)
    alive_log_probs = np.asarray(alive_log_probs, dtype=np.float32)
    fin_log_probs = np.asarray(fin_log_probs, dtype=np.float32)
    still_prompt = np.asarray(still_prompt).astype(np.float32).reshape(P_FULL, 1)
    is_first = np.asarray(is_first).astype(np.float32).reshape(P_FULL, 1)
    cp = int(np.asarray(cur_pos))

    in_maps = []
    for c in range(N_CORES):
        s = slice(c * Pc, (c + 1) * Pc)
        seqs = np.concatenate([alive_seq[s].reshape(R, S),
                               fin_seq[s].reshape(R, S)], axis=0)
        in_maps.append({
            "probs": np.ascontiguousarray(probs[s].reshape(R, V)),
            "seqs": np.ascontiguousarray(seqs),
            "alive_lp": np.ascontiguousarray(alive_log_probs[s]),
            "fin_lp": np.ascontiguousarray(fin_log_probs[s]),
            "sp": np.ascontiguousarray(still_prompt[s]),
            "isf": np.ascontiguousarray(is_first[s]),
        })
    return in_maps


def assemble_outputs(results):
    att = np.concatenate([r["att"] for r in results], axis=0).astype(np.int32)
    na_seq = np.concatenate(
        [r["na_seq"].reshape(Pc, D, S) for r in results], axis=0).astype(np.int32)
    na_lp = np.concatenate([r["na_lp"] for r in results], axis=0).astype(np.float32)
    nf_seq = np.concatenate(
        [r["nf_seq"].reshape(Pc, D, S) for r in results], axis=0).astype(np.int32)
    nf_lp = np.concatenate([r["nf_lp"] for r in results], axis=0).astype(np.float32)
    return (att, na_seq, na_lp, nf_seq, nf_lp)


def kernel(probs, alive_seq, fin_seq, alive_log_probs, fin_log_probs,
           still_prompt, is_first, cur_pos, _trace=False, _trace_kwargs=None):
    from concourse.bass_utils import run_bass_kernel_spmd

    nc = _get_nc(cur_pos)
    in_maps = make_in_maps(probs, alive_seq, fin_seq, alive_log_probs,
                           fin_log_probs, still_prompt, is_first, cur_pos)
    res = run_bass_kernel_spmd(
        nc, in_maps, core_ids=list(range(N_CORES)), trace=_trace,
        **(_trace_kwargs or {}))
    out = assemble_outputs(res.results)
    if _trace:
        return out, res
    return out


# revision 28
# speedup vs baseline: 1.0483x; 1.0483x over previous
"""Trainium2 Bass kernel for beam-search top-k masking (nn_Beam_57612691308621).

Strategy: shard the prompt dim P=32 across 8 NeuronCores (4 prompts each).
Each core, fully on-device:
  1. Streams its (32 rows x 128000) probs shard through SBUF in chunks,
     computing per-128-element segment maxima (the only full-data pass),
     transposed per-chunk into a per-beam-row layout.
  2. Selects top-16 segments per beam row (max8/max_index/match_replace),
     re-gathers those segments from HBM via indirect DMA, and extracts the
     exact per-beam top-16 prob values + vocab indices.
  3. Computes log-probs (ScalarE Ln) for the 16 candidates/beam only
     (log is monotone, so per-beam ordering by prob == ordering by logprob),
     then does the joint (beam, vocab) top-16 per prompt, the first-step
     override, EOS masking, grow-alive / grow-fin top-8, gathers the output
     sequence rows on device, and writes the new token at cur_pos via a
     bounds-checked indirect scatter.
Host only shards inputs / concatenates outputs.
"""

import os
import sys

for _p in ("/opt/trn_rl_repo", "/root/.axon_site", "/root/.axon_site/_ro/trn_rl_repo",
           "/root/.axon_site/_ro/pypackages"):
    if os.path.isdir(_p) and _p not in sys.path:
        sys.path.append(_p)

import numpy as np

import concourse.bass as bass
import concourse.bacc as bacc
import concourse.mybir as mybir
from concourse import tile

dt = mybir.dt
AF = mybir.ActivationFunctionType
ALU = mybir.AluOpType
AX = mybir.AxisListType

N_CORES = 8
P_FULL, D, V, S = 32, 8, 128000, 2048
Pc = P_FULL // N_CORES          # prompts per core = 4
R = Pc * D                      # beam rows per core = 32
Q = 4                           # row quarters -> R*Q = 128 partitions
Vq = V // Q                     # 32000 elems per partition
W = 128                         # segment width
NSEG = Vq // W                  # 250 segments per partition
NSEG_ROW = V // W               # 1000 segments per beam row
NCHUNK = 10
CW = Vq // NCHUNK               # 3200 elems per chunk per partition
SEG_PER_CHUNK = CW // W         # 25
K2 = 16                         # 2*D candidates
EOS = 2
INF = 1.0e7
NEG = -3.0e38
OOB = 10 ** 9                   # scatter offset sentinel (dropped by bounds check)


def build_core_program(nc, cur_pos):
    f32, i32, u32, u8 = dt.float32, dt.int32, dt.uint32, dt.uint8

    probs = nc.dram_tensor("probs", (R, V), f32, kind="ExternalInput")
    seqs = nc.dram_tensor("seqs", (2 * R, S), i32, kind="ExternalInput")
    alive_lp = nc.dram_tensor("alive_lp", (Pc, D), f32, kind="ExternalInput")
    fin_lp = nc.dram_tensor("fin_lp", (Pc, D), f32, kind="ExternalInput")
    sp_in = nc.dram_tensor("sp", (Pc, 1), f32, kind="ExternalInput")
    isf_in = nc.dram_tensor("isf", (Pc, 1), f32, kind="ExternalInput")

    att_out = nc.dram_tensor("att", (Pc, D), i32, kind="ExternalOutput")
    na_seq_out = nc.dram_tensor("na_seq", (R, S), i32, kind="ExternalOutput")
    na_lp_out = nc.dram_tensor("na_lp", (Pc, D), f32, kind="ExternalOutput")
    nf_seq_out = nc.dram_tensor("nf_seq", (R, S), i32, kind="ExternalOutput")
    nf_lp_out = nc.dram_tensor("nf_lp", (Pc, D), f32, kind="ExternalOutput")

    with tile.TileContext(nc) as tc:
        with (
            tc.tile_pool(name="stream", bufs=4) as stream_pool,
            tc.tile_pool(name="work", bufs=1) as wp,
            tc.tile_pool(name="stage", bufs=1) as sgp,
        ):
            # ------- phase 1 first: keep the SP HWDGE queue clear for streaming
            probs_t = probs.ap().rearrange("r (q v) -> (r q) v", q=Q)  # (128, 32000)
            segmax = wp.tile([128, NSEG], f32)
            segrow = wp.tile([R, NSEG_ROW], f32)    # per-beam-row segment maxima
            # ramped chunk sizes: small first chunks start the DVE pipeline early
            chunk_ws = [1280, 1920, 2560, 3200] + [3840] * 6
            assert sum(chunk_ws) == Vq
            col = 0
            for cw in chunk_ws:
                chunk = stream_pool.tile([128, max(chunk_ws)], f32, tag="chunk")
                nc.sync.dma_start(chunk[:, 0:cw], probs_t[:, col:col + cw])
                s0, s1 = col // W, (col + cw) // W
                seg_sl = segmax[:, s0:s1]
                nc.vector.reduce_max(
                    seg_sl.unsqueeze(2),
                    chunk[:, 0:cw].rearrange("p (s w) -> p s w", w=W),
                    axis=AX.X,
                )
                # transpose this chunk's maxima into row-major layout (ACT queue)
                dst = segrow[:].rearrange("r (q s) -> r q s", q=Q)[:, :, s0:s1]
                nc.scalar.dma_start(dst, seg_sl)
                col += cw

            # ---------------- constants (inline tables, ACT queue) -----------
            rid = np.arange(R)
            c32_np = np.concatenate([
                rid[:, None],                                   # 0: row id
                (rid * NSEG_ROW)[:, None],                      # 1: row seg base
                np.repeat(np.arange(K2)[None, :], R, axis=0),   # 2..17: slot iota
                (rid % D)[:, None],                             # 18: beam-slot d
                ((rid // D) * D)[:, None],                      # 19: prompt row base
                np.ones((R, 1)),                                # 20: ones
            ], axis=1).astype(np.float32)                       # (32, 21)
            c32_dram = nc.inline_tensor(c32_np, name="c32")
            c32 = wp.tile([R, 5 + K2], f32)
            nc.scalar.dma_start(c32[:], c32_dram.ap())
            rowid_f = c32[:, 0:1]
            rowbase_f = c32[:, 1:2]
            islot_f = c32[:, 2:2 + K2]
            dmod_f = c32[:, 2 + K2:3 + K2]
            rowbase8_f = c32[:, 3 + K2:4 + K2]
            ones32 = c32[:, 4 + K2:5 + K2]

            c64_np = np.repeat(np.arange(D * K2)[None, :], 2 * R, axis=0).astype(np.float32)
            c64_dram = nc.inline_tensor(c64_np, name="c64")
            c64 = wp.tile([2 * R, D * K2], f32)
            nc.scalar.dma_start(c64[:], c64_dram.ap())

            # ---------------- small input loads (ACT queue) ------------------
            alive32 = wp.tile([R, 1], f32)          # alive_lp per beam row
            nc.scalar.dma_start(alive32[:], alive_lp.ap().rearrange("p d -> (p d)").unsqueeze(1))
            alive8 = wp.tile([Pc, D], f32)
            nc.scalar.dma_start(alive8[:], alive_lp.ap())
            fin8 = wp.tile([Pc, D], f32)
            nc.scalar.dma_start(fin8[:], fin_lp.ap())
            sp1 = wp.tile([Pc, 1], f32)
            nc.scalar.dma_start(sp1[:], sp_in.ap())
            isf1 = wp.tile([Pc, 1], f32)
            nc.scalar.dma_start(isf1[:], isf_in.ap())
            sp_u8 = wp.tile([Pc, 1], u8)
            nc.vector.tensor_copy(sp_u8[:], sp1[:])
            isf_u8 = wp.tile([Pc, 1], u8)
            nc.vector.tensor_copy(isf_u8[:], isf1[:])

            # ---------------- phase 2: top-16 segments per beam row ----------
            segv = wp.tile([R, K2], f32)
            segidx = wp.tile([R, K2], u32)
            sm2 = wp.tile([R, NSEG_ROW], f32)
            nc.vector.max(out=segv[:, 0:8], in_=segrow[:])
            nc.vector.max_index(out=segidx[:, 0:8], in_max=segv[:, 0:8], in_values=segrow[:])
            nc.vector.match_replace(out=sm2[:], in_to_replace=segv[:, 0:8],
                                    in_values=segrow[:], imm_value=NEG)
            nc.vector.max(out=segv[:, 8:16], in_=sm2[:])
            nc.vector.max_index(out=segidx[:, 8:16], in_max=segv[:, 8:16], in_values=sm2[:])

            lseg_f = wp.tile([R, K2], f32)          # per-row local seg id (0..999)
            gseg_f = wp.tile([R, K2], f32)          # global seg id = row*1000 + local
            gseg_i = wp.tile([R, K2], i32)
            for h in range(2):
                hs = slice(8 * h, 8 * h + 8)
                nc.vector.tensor_copy(lseg_f[:, hs], segidx[:, hs])
                nc.vector.tensor_scalar(gseg_f[:, hs], lseg_f[:, hs],
                                        rowbase_f[:, 0:1], None, op0=ALU.add)
                nc.vector.tensor_copy(gseg_i[:, hs], gseg_f[:, hs])

            # ---------------- phase 3: drill-down gather + exact row top-16 --
            # distribute the 512 segment fetches over all 128 partitions:
            # partition (r, g) on call c fetches the row's slot s = c*4+g
            # partition (r,g) on call c fetches rank slot s = c*4+g, so calls 0/1
            # only need the first max8 round's results (pipelines with round 2)
            gsegP = wp.tile([R, K2], i32)           # [g*4+c] = gseg[c*4+g]
            gsegT = wp.tile([4 * R, Q], i32)
            gathQ = wp.tile([4 * R, Q * W], f32)
            gath = wp.tile([R, K2 * W], f32)        # [r, (g*4+c)*W + w]
            gath_v = gath[:].rearrange("r (g c w) -> r g c w", g=Q, c=Q)
            probs_seg = probs.ap().rearrange("r (s w) -> (r s) w", w=W)
            gsegP_v = gsegP[:].rearrange("r (g c) -> r g c", g=Q)
            gseg_v = gseg_i[:].rearrange("r (c g) -> r g c", g=Q)
            for h in range(2):
                nc.vector.tensor_copy(gsegP_v[:, :, 2 * h:2 * h + 2],
                                      gseg_v[:, :, 2 * h:2 * h + 2])
                nc.scalar.dma_start(
                    gsegT[:, 2 * h:2 * h + 2],
                    gsegP_v[:, :, 2 * h:2 * h + 2])
                for c in (2 * h, 2 * h + 1):
                    nc.gpsimd.indirect_dma_start(
                        out=gathQ[:, c * W:(c + 1) * W],
                        out_offset=None,
                        in_=probs_seg,
                        in_offset=bass.IndirectOffsetOnAxis(ap=gsegT[:, c:c + 1], axis=0),
                    )
                    nc.sync.dma_start(gath_v[:, :, c, :],
                                      gathQ[:, c * W:(c + 1) * W])
            bv = wp.tile([R, K2], f32)              # per-beam-row top-16 prob values
            qpos = wp.tile([R, K2], u32)            # positions in gath (0..2047)
            g2 = wp.tile([R, K2 * W], f32)
            nc.vector.max(out=bv[:, 0:8], in_=gath[:])
            nc.vector.max_index(out=qpos[:, 0:8], in_max=bv[:, 0:8], in_values=gath[:])
            nc.vector.match_replace(out=g2[:], in_to_replace=bv[:, 0:8],
                                    in_values=gath[:], imm_value=NEG)
            nc.vector.max(out=bv[:, 8:16], in_=g2[:])
            nc.vector.max_index(out=qpos[:, 8:16], in_max=bv[:, 8:16], in_values=g2[:])

            slot_u = wp.tile([R, K2], u32)
            nc.vector.tensor_scalar(slot_u[:], qpos[:], 7, None, op0=ALU.logical_shift_right)
            off_u = wp.tile([R, K2], u32)
            nc.vector.tensor_scalar(off_u[:], qpos[:], W - 1, None, op0=ALU.bitwise_and)
            slot_f = wp.tile([R, K2], f32)
            nc.vector.tensor_copy(slot_f[:], slot_u[:])
            off_f = wp.tile([R, K2], f32)
            nc.vector.tensor_copy(off_f[:], off_u[:])

            # local seg ids keyed by gath slot position (rank slot c*4+g)
            lperm_f = wp.tile([R, K2], f32)
            nc.vector.tensor_copy(
                lperm_f[:].rearrange("r (g c) -> r g c", g=Q),
                lseg_f[:].rearrange("r (c g) -> r g c", g=Q))
            # one-hot over the 16 slot positions
            eq3 = wp.tile([R, K2 * K2], f32)
            eq3v = eq3[:].rearrange("p (c s) -> p c s", s=K2)
            nc.vector.tensor_tensor(
                eq3v,
                slot_f[:].unsqueeze(2).broadcast_to([R, K2, K2]),
                islot_f.unsqueeze(1).broadcast_to([R, K2, K2]),
                op=ALU.is_equal,
            )
            nc.vector.tensor_tensor(
                eq3v, eq3v,
                lperm_f[:].unsqueeze(1).broadcast_to([R, K2, K2]),
                op=ALU.mult,
            )
            lcand_f = wp.tile([R, K2], f32)
            nc.vector.reduce_sum(lcand_f[:].unsqueeze(2), eq3v, axis=AX.X)

            btok = wp.tile([R, K2], f32)            # vocab index per beam candidate
            nc.vector.tensor_scalar(btok[:], lcand_f[:], float(W), None, op0=ALU.mult)
            nc.vector.tensor_add(btok[:], btok[:], off_f[:])

            # ---------------- phase 5: log + joint top-16 per prompt ---------
            logp = wp.tile([R, K2], f32)
            nc.scalar.activation(logp[:], bv[:], AF.Ln)
            curr = wp.tile([R, K2], f32)
            nc.vector.tensor_scalar(curr[:], logp[:], alive32[:, 0:1], None, op0=ALU.add)

            currp = wp.tile([Pc, D * K2], f32)      # (4, 128) joint candidates
            nc.sync.dma_start(currp[:].rearrange("p (d c) -> p d c", d=D), curr[:])
            tokp = wp.tile([Pc, D * K2], f32)
            nc.sync.dma_start(tokp[:].rearrange("p (d c) -> p d c", d=D), btok[:])

            jv = wp.tile([Pc, K2], f32)             # joint top-16 logprob values
            jpos = wp.tile([Pc, K2], u32)
            j2 = wp.tile([Pc, D * K2], f32)
            nc.vector.max(out=jv[:, 0:8], in_=currp[:])
            nc.vector.max_index(out=jpos[:, 0:8], in_max=jv[:, 0:8], in_values=currp[:])
            nc.vector.match_replace(out=j2[:], in_to_replace=jv[:, 0:8],
                                    in_values=currp[:], imm_value=NEG)
            nc.vector.max(out=jv[:, 8:16], in_=j2[:])
            nc.vector.max_index(out=jpos[:, 8:16], in_max=jv[:, 8:16], in_values=j2[:])

            jpos_f = wp.tile([Pc, K2], f32)
            nc.vector.tensor_copy(jpos_f[:], jpos[:])
            beam_u = wp.tile([Pc, K2], u32)
            nc.vector.tensor_scalar(beam_u[:], jpos[:], 4, None, op0=ALU.logical_shift_right)
            bj = wp.tile([Pc, 2 * K2], f32)         # [beam(16) | jtok(16)]
            beam_f = bj[:, 0:K2]                    # parent beam of each topk entry
            nc.vector.tensor_copy(beam_f, beam_u[:])

            # token of each joint-topk entry: one-hot dot in (64,1) layout
            jposT = wp.tile([2 * R, 1], f32)
            nc.scalar.dma_start(jposT[:], jpos_f[:])
            tokpT = wp.tile([2 * R, D * K2], f32)
            nc.scalar.dma_start(
                tokpT[:], tokp[:].unsqueeze(1).broadcast_to([Pc, K2, D * K2]))
            junk64 = wp.tile([2 * R, D * K2], f32)
            jtokT = wp.tile([2 * R, 1], f32)
            nc.vector.scalar_tensor_tensor(
                out=junk64[:], in0=c64[:], scalar=jposT[:, 0:1], in1=tokpT[:],
                op0=ALU.is_equal, op1=ALU.mult, accum_out=jtokT[:])
            jtok = bj[:, K2:2 * K2]                 # token of each topk entry
            nc.scalar.dma_start(jtok, jtokT[:])

            # ---------------- phase 6: first-generation override -------------
            isfb = isf_u8[:, 0:1].broadcast_to([Pc, K2])
            nc.vector.copy_predicated(jv[:], isfb, currp[:, 0:K2])
            nc.vector.copy_predicated(jtok, isfb, tokp[:, 0:K2])

            # ---------------- phase 7: grow_alive / grow_fin -----------------
            fino = wp.tile([Pc, K2], f32)           # finished = tok == EOS
            nc.vector.tensor_scalar(fino[:], jtok, float(EOS), None, op0=ALU.is_equal)

            am = wp.tile([Pc, K2], f32)             # alive-masked = jv + fino*(-INF)
            nc.vector.scalar_tensor_tensor(
                out=am[:], in0=fino[:], scalar=-INF, in1=jv[:],
                op0=ALU.mult, op1=ALU.add)

            nav = wp.tile([Pc, D], f32)
            np2 = wp.tile([Pc, 2 * D], u32)         # [nap(8) | nfp(8)]
            nc.vector.max(out=nav[:], in_=am[:])
            nc.vector.max_index(out=np2[:, 0:2 * D:2], in_max=nav[:], in_values=am[:])

            spb8 = sp_u8[:, 0:1].broadcast_to([Pc, D])

            # new_alive_lp = where(sp, alive_lp, nav)
            nalp = wp.tile([Pc, D], f32)
            nc.vector.tensor_copy(nalp[:], nav[:])
            nc.vector.copy_predicated(nalp[:], spb8, alive8[:])
            nc.sync.dma_start(na_lp_out.ap(), nalp[:])

            # ----- fin side (per-prompt layout) -----
            notf = wp.tile([Pc, K2], f32)
            nc.vector.tensor_scalar(notf[:], fino[:], -1.0, 1.0, op0=ALU.mult, op1=ALU.add)
            fm = wp.tile([Pc, K2], f32)             # fin-masked = jv + (1-fino)*(-INF)
            nc.vector.scalar_tensor_tensor(
                out=fm[:], in0=notf[:], scalar=-INF, in1=jv[:],
                op0=ALU.mult, op1=ALU.add)

            cat = wp.tile([Pc, D + K2], f32)        # [fin_lp(8) | fin_masked(16)]
            nc.vector.tensor_copy(cat[:, 0:D], fin8[:])
            nc.vector.tensor_copy(cat[:, D:D + K2], fm[:])

            nfv = wp.tile([Pc, D], f32)
            nc.vector.max(out=nfv[:], in_=cat[:])
            nc.vector.max_index(out=np2[:, 1:2 * D:2], in_max=nfv[:], in_values=cat[:])
            np2_f = wp.tile([Pc, 2 * D], f32)
            nc.vector.tensor_copy(np2_f[:], np2[:])

            # new_fin_lp = where(sp, fin_lp, nfv)
            nflp = wp.tile([Pc, D], f32)
            nc.vector.tensor_copy(nflp[:], nfv[:])
            nc.vector.copy_predicated(nflp[:], spb8, fin8[:])
            nc.sync.dma_start(nf_lp_out.ap(), nflp[:])

            # ----- per-(prompt, slot) layout: one output row per partition -----
            npT = wp.tile([R, 2], f32)              # [nap | nfp] per row
            nc.scalar.dma_start(npT[:],
                                np2_f[:].rearrange("p (d f) -> p d f", f=2))
            napT = npT[:, 0:1]
            nfpT = npT[:, 1:2]
            bjT = wp.tile([R, 2 * K2], f32)         # [beam(16) | jtok(16)] per row
            nc.scalar.dma_start(
                bjT[:], bj[:].unsqueeze(1).broadcast_to([Pc, D, 2 * K2]))
            beamT = bjT[:, 0:K2]
            jtokrT = bjT[:, K2:2 * K2]
            spT = wp.tile([R, 1], f32)
            nc.scalar.dma_start(
                spT[:], sp1[:].unsqueeze(1).broadcast_to([Pc, D, 1]))
            spT_u8 = wp.tile([R, 1], u8)
            nc.vector.tensor_copy(spT_u8[:], spT[:])

            junk32 = wp.tile([R, K2], f32)
            attT = wp.tile([R, 1], f32)             # selected parent beam per row
            nc.vector.scalar_tensor_tensor(
                out=junk32[:], in0=islot_f, scalar=napT, in1=beamT,
                op0=ALU.is_equal, op1=ALU.mult, accum_out=attT[:])
            atokT = wp.tile([R, 1], f32)            # new token per alive row
            nc.vector.scalar_tensor_tensor(
                out=junk32[:], in0=islot_f, scalar=napT, in1=jtokrT,
                op0=ALU.is_equal, op1=ALU.mult, accum_out=atokT[:])

            # attention_change_ids = where(sp, d, att)
            nc.vector.copy_predicated(attT[:], spT_u8[:], dmod_f)
            att_i = wp.tile([R, 1], i32)
            nc.vector.tensor_copy(att_i[:], attT[:])
            nc.sync.dma_start(att_out.ap(), att_i[:])

            # alive source row + token + write flag
            arow_f = wp.tile([R, 1], f32)
            nc.vector.tensor_scalar(arow_f[:], attT[:], rowbase8_f[:, 0:1], None,
                                    op0=ALU.add)
            notspT = wp.tile([R, 1], f32)
            nc.vector.tensor_scalar(notspT[:], spT[:], -1.0, 1.0,
                                    op0=ALU.mult, op1=ALU.add)

            # fin old/new resolution per row
            isoldT = wp.tile([R, 1], f32)           # nfp < 8 (or sp)
            nc.vector.tensor_scalar(isoldT[:], nfpT, float(D), None, op0=ALU.is_lt)
            nc.vector.copy_predicated(isoldT[:], spT_u8[:], ones32)
            isoldT_u8 = wp.tile([R, 1], u8)
            nc.vector.tensor_copy(isoldT_u8[:], isoldT[:])
            oldrowT = wp.tile([R, 1], f32)
            nc.vector.tensor_scalar(oldrowT[:], nfpT, float(D - 1), None, op0=ALU.min)
            nc.vector.copy_predicated(oldrowT[:], spT_u8[:], dmod_f)
            candT = wp.tile([R, 1], f32)            # clamp(nfp-8, 0)
            nc.vector.tensor_scalar(candT[:], nfpT, float(D), 0.0,
                                    op0=ALU.subtract, op1=ALU.max)

            fbeamT = wp.tile([R, 1], f32)
            nc.vector.scalar_tensor_tensor(
                out=junk32[:], in0=islot_f, scalar=candT[:, 0:1], in1=beamT,
                op0=ALU.is_equal, op1=ALU.mult, accum_out=fbeamT[:])
            ftokT = wp.tile([R, 1], f32)
            nc.vector.scalar_tensor_tensor(
                out=junk32[:], in0=islot_f, scalar=candT[:, 0:1], in1=jtokrT,
                op0=ALU.is_equal, op1=ALU.mult, accum_out=ftokT[:])

            # fin source row: old -> R + base + oldrow ; new -> base + fbeam
            frow_f = wp.tile([R, 1], f32)
            nc.vector.tensor_scalar(frow_f[:], fbeamT[:], rowbase8_f[:, 0:1], None,
                                    op0=ALU.add)
            frow_old = wp.tile([R, 1], f32)
            nc.vector.tensor_scalar(frow_old[:], oldrowT[:], float(R),
                                    rowbase8_f[:, 0:1], op0=ALU.add, op1=ALU.add)
            nc.vector.copy_predicated(frow_f[:], isoldT_u8[:], frow_old[:])

            # fin write flag = !sp & !isold
            wf_f = wp.tile([R, 1], f32)
            nc.vector.scalar_tensor_tensor(
                out=wf_f[:], in0=isoldT[:], scalar=0.0, in1=notspT[:],
                op0=ALU.is_equal, op1=ALU.mult)

            # ---------------- seq gathers + token writes + outputs -----------
            # combined offsets/tokens/flags on 64 partitions: [alive rows | fin rows]
            offs = wp.tile([2 * R, 1], i32)
            nc.vector.tensor_copy(offs[0:R, :], arow_f[:])
            nc.vector.tensor_copy(offs[R:2 * R, :], frow_f[:])
            toks = wp.tile([2 * R, 1], i32)
            nc.vector.tensor_copy(toks[0:R, :], atokT[:])
            nc.vector.tensor_copy(toks[R:2 * R, :], ftokT[:])
            wmask = wp.tile([2 * R, 1], u8)
            nc.vector.tensor_copy(wmask[0:R, :], notspT[:])
            nc.vector.tensor_copy(wmask[R:2 * R, :], wf_f[:])

            stage = sgp.tile([2 * R, S], i32)
            nc.gpsimd.indirect_dma_start(
                out=stage[:], out_offset=None, in_=seqs.ap(),
                in_offset=bass.IndirectOffsetOnAxis(ap=offs[:], axis=0))

            # write new token at the cur_pos column of the staged rows
            # (cur_pos is specialized at compile time; recompiled if it changes)
            cp = int(cur_pos)
            nc.vector.copy_predicated(stage[:, cp:cp + 1], wmask[:], toks[:])

            nc.sync.dma_start(na_seq_out.ap(), stage[0:R, :])
            nc.sync.dma_start(nf_seq_out.ap(), stage[R:2 * R, :])

    return nc


_CACHED_NC = {}


def _get_nc(cur_pos=1024):
    cp = int(cur_pos)
    if cp not in _CACHED_NC:
        nc = bacc.Bacc("TRN2", target_bir_lowering=False, debug=False)
        build_core_program(nc, cp)
        nc.compile()
        _CACHED_NC[cp] = nc
    return _CACHED_NC[cp]


def make_in_maps(probs, alive_seq, fin_seq, alive_log_probs, fin_log_probs,
                 still_prompt, is_first, cur_pos):
    probs = np.asarray(probs, dtype=np.float32).reshape(P_FULL, D, V)
    alive_seq = np.asarray(alive_seq, dtype=np.int32)
    fin_seq = np.asarray(fin_seq, dtype=np.int32)
    alive_log_probs = np.asarray(alive_log_probs, dtype=np.float32)
    fin_log_probs = np.asarray(fin_log_probs, dtype=np.float32)
    still_prompt = np.asarray(still_prompt).astype(np.float32).reshape(P_FULL, 1)
    is_first = np.asarray(is_first).astype(np.float32).reshape(P_FULL, 1)
    cp = int(np.asarray(cur_pos))

    in_maps = []
    for c in range(N_CORES):
        s = slice(c * Pc, (c + 1) * Pc)
        seqs = np.concatenate([alive_seq[s].reshape(R, S),
                               fin_seq[s].reshape(R, S)], axis=0)
        in_maps.append({
            "probs": np.ascontiguousarray(probs[s].reshape(R, V)),
            "seqs": np.ascontiguousarray(seqs),
            "alive_lp": np.ascontiguousarray(alive_log_probs[s]),
            "fin_lp": np.ascontiguousarray(fin_log_probs[s]),
            "sp": np.ascontiguousarray(still_prompt[s]),
            "isf": np.ascontiguousarray(is_first[s]),
        })
    return in_maps


def assemble_outputs(results):
    att = np.concatenate([r["att"] for r in results], axis=0).astype(np.int32)
    na_seq = np.concatenate(
        [r["na_seq"].reshape(Pc, D, S) for r in results], axis=0).astype(np.int32)
    na_lp = np.concatenate([r["na_lp"] for r in results], axis=0).astype(np.float32)
    nf_seq = np.concatenate(
        [r["nf_seq"].reshape(Pc, D, S) for r in results], axis=0).astype(np.int32)
    nf_lp = np.concatenate([r["nf_lp"] for r in results], axis=0).astype(np.float32)
    return (att, na_seq, na_lp, nf_seq, nf_lp)


def kernel(probs, alive_seq, fin_seq, alive_log_probs, fin_log_probs,
           still_prompt, is_first, cur_pos, _trace=False, _trace_kwargs=None):
    from concourse.bass_utils import run_bass_kernel_spmd

    nc = _get_nc(cur_pos)
    in_maps = make_in_maps(probs, alive_seq, fin_seq, alive_log_probs,
                           fin_log_probs, still_prompt, is_first, cur_pos)
    res = run_bass_kernel_spmd(
        nc, in_maps, core_ids=list(range(N_CORES)), trace=_trace,
        **(_trace_kwargs or {}))
    out = assemble_outputs(res.results)
    if _trace:
        return out, res
    return out
